# revision 15
# baseline (speedup 1.0000x reference)
"""Trainium2 Bass kernel for the deterministic legality module.

Computes, for each board b, filter f and top-left placement (i,j):
    legal[b,f,i,j] = 1.0 iff every occupied cell of filter f, placed at
    (i,j), lands in-bounds on a free cell of board b (and f is non-empty).

Two structural reductions over the dense formulation:

1. Feasibility pruning: a filter whose max occupied row is r and max
   occupied col is c can only be legal at the (9-r)*(9-c) top-left
   positions where its footprint stays in bounds -- every other (f,p)
   column of the output is constant zero (~68% of them).  Only feasible
   columns are computed on device; the host scatters them back.

2. Pair packing: two placements (same filter) share one matmul column
   with weights geo(p0) + 32*geo(p1) (entries {0,1,32,33}, exact in
   bf16).  The accumulator A = corr0 + 32*corr1 <= 825 is exact in f32
   and in the fp16 output; corr_i <= area_i <= 25 < 32 so the fields
   never interfere.  The host decodes corr_i = (A >> 5i) & 31 and
   compares with area_i.  This halves PE columns and, critically, the
   PSUM->SBUF drain (the PSUM read port of DVE+ACT is the pipeline
   bottleneck), at unchanged HBM store bytes (1 byte per placement).

Loop order is column-group-major with the 4 batch blocks inner, so one
uploaded M slab feeds 4 matmuls before the next slab's completion
semaphore is needed -- DMA completion latency is ~2.5us and would
otherwise starve the PE during the ramp (measured).  The output DRAM
layout is [128, 4*npair] (partition = board-in-block, free =
(block, col)) so per-block staging tiles store contiguously.

Pipeline per core: bf16 matmul (K=81 padded to 128 partitions, N<=512)
-> PSUM ring (4 slots of 1024 f32 cols) -> f32->fp16 copy drain split
across DVE/ACT by greedy time balance ((120+FD)/0.96GHz vs
(172+FD)/1.2GHz) -> per-block SBUF staging [128, 2048] -> HBM store on
the SP HWDGE ring.  M slabs upload on the SP ring, boardT on the ACT
ring.  Warmup matmuls on memset zeros keep the PE busy from the end of
the framework preamble until boardT's completion fires (~10.4us), so
the HAM clock gate (~4.2us of continuous PE activity at half clock)
lifts as early as possible; dummy matmuls at the end hold it through
the drain/store tail.
"""

import numpy as np
import ml_dtypes

N_CORES = 8
BATCH = 4096
BPC = BATCH // N_CORES  # 512 boards per core
NPOS = 81               # 9x9 board cells / placements
NF = 264                # filters
NCOL = NF * NPOS        # full output columns per board
KPAD = 128              # uploads padded to 128 partitions for DMA fan-out
PACK = 32               # field base: A = corr0 + 32*corr1
NKB = 4                 # batch blocks of 128 boards

COL_TILE = 512          # one PSUM bank of f32
GRP = 1024              # PSUM ring slot / one drain op
STAGE = 2048            # per-block staging tile / store DMA granularity
_DVE_NS = lambda fd: (120.0 + fd) / 0.96
_ACT_NS = lambda fd: (172.0 + fd) / 1.2


def _plan_cols(filters: np.ndarray):
    """Pair feasible placements per filter.

    Returns (npair, c0_idx, c1_idx, c1_valid) with col1 == -1 marking a
    dummy second half (odd feasible count).
    """
    F = np.asarray(filters, dtype=np.float32).reshape(NF, 5, 5) > 0.5
    pairs = []
    for f in range(NF):
        occ = F[f]
        if not occ.any():
            continue
        rmax = int(np.where(occ.any(axis=1))[0].max())
        cmax = int(np.where(occ.any(axis=0))[0].max())
        cols = [f * NPOS + i * 9 + j
                for i in range(9 - rmax) for j in range(9 - cmax)]
        for k in range(0, len(cols) - 1, 2):
            pairs.append((cols[k], cols[k + 1]))
        if len(cols) % 2:
            pairs.append((cols[-1], -1))
    c0 = np.asarray([p[0] for p in pairs], dtype=np.int64)
    c1 = np.asarray([p[1] for p in pairs], dtype=np.int64)
    return c0, c1, c1 >= 0


def _geo(filters: np.ndarray) -> np.ndarray:
    """geo[81, 264*81] f32: filter f placed at position p, flattened."""
    F = np.asarray(filters, dtype=np.float32).reshape(NF, 5, 5)
    G = np.zeros((NPOS, NF, NPOS), dtype=np.float32)
    for i in range(9):
        h = min(5, 9 - i)
        for j in range(9):
            w = min(5, 9 - j)
            blk = np.zeros((NF, 9, 9), dtype=np.float32)
            blk[:, i:i + h, j:j + w] = F[:, :h, :w]
            G[:, :, i * 9 + j] = blk.reshape(NF, NPOS).T
    return G.reshape(NPOS, NF * NPOS)


def _build_m(filters: np.ndarray, c0: np.ndarray, c1: np.ndarray,
             c1v: np.ndarray) -> np.ndarray:
    """M [128, npair] bf16: geo(c0) + 32*geo(c1)."""
    G = _geo(filters)
    M = np.zeros((KPAD, len(c0)), dtype=np.float32)
    M[:NPOS] = G[:, c0]
    M[:NPOS, c1v] += PACK * G[:, c1[c1v]]
    return M.astype(ml_dtypes.bfloat16)


def _build_boardt(board_free: np.ndarray) -> np.ndarray:
    """boardT [cores, 128, 512] bf16: transposed boards, zero padded."""
    b = np.asarray(board_free, dtype=np.float32).reshape(N_CORES, BPC, NPOS)
    bt = np.zeros((N_CORES, KPAD, BPC), dtype=np.float32)
    bt[:, :NPOS, :] = b.transpose(0, 2, 1)
    return bt.astype(ml_dtypes.bfloat16)


def _groups(npair: int):
    """Column groups, aligned 1:1 with the upload slabs.

    A small leading group (the first slab's completion semaphore gates
    the first real matmul) and a small trailing group (the final store
    should be tiny).  No group may span a slab boundary, or the PE
    stalls mid-group on the next slab's ~2.8us completion latency.
    """
    bounds = [0, 512, 1536]
    while npair - bounds[-1] > GRP + 512:
        bounds.append(bounds[-1] + GRP)
    if npair - bounds[-1] > 512:
        bounds.append(bounds[-1] + 512)
    bounds.append(npair)
    return [(b0, b1 - b0) for b0, b1 in zip(bounds[:-1], bounds[1:])]


def _drain_plan(npair: int):
    """Greedy DVE/ACT time-balanced [(g0, fd, kb, engine)] in issue order."""
    plan = []
    tv = ts = 0.0
    groups = _groups(npair)
    for gi, (g0, fd) in enumerate(groups):
        for kb in range(NKB):
            if gi == len(groups) - 1 and kb == NKB - 1:
                plan.append((g0, fd, kb, 'split'))
            elif tv + _DVE_NS(fd) <= ts + _ACT_NS(fd):
                tv += _DVE_NS(fd)
                plan.append((g0, fd, kb, 'v'))
            else:
                ts += _ACT_NS(fd)
                plan.append((g0, fd, kb, 's'))
    return plan


def _build_module(npair: int):
    import concourse.bass as bass
    import concourse.mybir as mybir
    import concourse.tile as tile

    f32 = mybir.dt.float32
    f16 = mybir.dt.float16
    bf16 = mybir.dt.bfloat16

    nc = bass.Bass("TRN2", target_bir_lowering=False, debug=False,
                   num_devices=N_CORES)

    boardt_d = nc.dram_tensor("boardt", [KPAD, BPC], bf16,
                              kind="ExternalInput")
    m_d = nc.dram_tensor("mmat", [KPAD, npair], bf16, kind="ExternalInput")
    # partition = board-in-block, free = (block, col)
    out_d = nc.dram_tensor("out", [128, NKB * npair], f16,
                           kind="ExternalOutput")

    plan = _drain_plan(npair)

    with tile.TileContext(nc) as tc:
        with tc.tile_pool(name="const", bufs=1) as cpool:
            boardT = cpool.tile([KPAD, BPC], bf16)
            msb = cpool.tile([KPAD, npair], bf16)

            # M slabs on the SP hwdge ring; each slab feeds 4 blocks of
            # matmuls, so ~1024-col slabs keep the PE fed through the
            # ~2.8us per-slab completion latency (small slab 0 so the
            # first real matmul starts as early as possible).  boardT
            # on the ACT ring in parallel.  Stores follow on SP.
            for s0, fd in _groups(npair):
                nc.sync.dma_start(msb[:, s0:s0 + fd], m_d[:, s0:s0 + fd])
            nc.scalar.dma_start(boardT[:], boardt_d[:])

            with (
                tc.tile_pool(name="wprep", bufs=1) as wprep,
                tc.tile_pool(name="psM", bufs=4, space="PSUM") as psM,
                tc.tile_pool(name="ostage", bufs=2) as ostage,
            ):
                # warm-up matmuls on memset zeros: PE-busy from the end
                # of the framework preamble (~7.8us) until boardT's
                # completion semaphore fires (~10.4us), so the HAM gate
                # qualification window starts early.  256-col pieces
                # keep the overshoot past the first real matmul small.
                wz = wprep.tile([128, 256], bf16, tag="wz")
                nc.vector.memset(wz[:], 0.0)
                wps = psM.tile([128, GRP], f32, tag="mm")
                for _ in range(11):
                    nc.tensor.matmul(wps[:, 0:256], wz[:, 0:128], wz[:],
                                     start=True, stop=True)
                for _ in range(3):
                    nc.tensor.matmul(wps[:, 0:128], wz[:, 0:128],
                                     wz[:, 0:128], start=True, stop=True)

                stages = {}   # kb -> (tile, s0)
                tails = []

                def _flush(kb, hi):
                    ot, s0 = stages.pop(kb)
                    nc.sync.dma_start(
                        out_d[:, kb * npair + s0:kb * npair + hi],
                        ot[:, :hi - s0])

                for (g0, fd, kb, eng) in plan:
                    if kb in stages and g0 + fd - stages[kb][1] > STAGE:
                        _flush(kb, g0)
                    if kb not in stages:
                        st_tile = ostage.tile([128, STAGE], f16,
                                              tag=f"ot{kb}", name=f"ot{kb}")
                        stages[kb] = (st_tile, g0)
                    ot, s0 = stages[kb]
                    lhsT = boardT[:, kb * 128:(kb + 1) * 128]
                    pt = psM.tile([128, GRP], f32, tag="mm")
                    for q in range(0, fd, COL_TILE):
                        w = min(COL_TILE, fd - q)
                        nc.tensor.matmul(pt[:, q:q + w], lhsT,
                                         msb[:, g0 + q:g0 + q + w],
                                         start=True, stop=True)
                    o0 = g0 - s0
                    if eng == 'v':
                        nc.vector.tensor_scalar_max(
                            ot[:, o0:o0 + fd], pt[:, :fd], 0.0)
                    elif eng == 's':
                        nc.scalar.activation(
                            ot[:, o0:o0 + fd], pt[:, :fd],
                            mybir.ActivationFunctionType.Copy)
                    else:  # final item: drain on both engines so the
                        # closing store starts as early as possible
                        hh = fd // 2
                        nc.vector.tensor_scalar_max(
                            ot[:, o0:o0 + hh], pt[:, :hh], 0.0)
                        nc.scalar.activation(
                            ot[:, o0 + hh:o0 + fd], pt[:, hh:fd],
                            mybir.ActivationFunctionType.Copy)
                    if g0 + GRP * 2 >= npair:
                        tails.append(pt)
                    if g0 + fd >= npair:
                        _flush(kb, npair)
                # dummy matmuls into already-drained tail slots: keep
                # the PE busy so the HAM clock gate stays lifted while
                # the last drains and stores run.
                for pt in tails:
                    for _ in range(2):
                        nc.tensor.matmul(pt[:, 0:256], wz[:, 0:128],
                                         wz[:], start=True, stop=True)
    return nc


def _legalize_multiwait(nc):
    """Split multi-wait instructions for this walrus build.

    The TPB instruction encodings carry exactly one semaphore wait, and
    the walrus codegen here refuses instructions with more ("Too many
    sync wait commands").  Hoist all but one wait onto EventSemaphore
    carrier instructions placed immediately before, on the same engine --
    the sequencer blocks on each carrier first, which is semantically
    identical.
    """
    import concourse.mybir as mybir

    for func in nc.m.functions:
        for blk in func.blocks:
            out = []
            changed = False
            for inst in blk.instructions:
                si = inst.sync_info
                waits = list(si.on_wait) if si is not None and si.on_wait else []
                if len(waits) > 1:
                    for j, w in enumerate(waits[:-1]):
                        carrier = mybir.InstEventSemaphore(
                            name=f"{inst.name}-xw{j}",
                            engine=inst.engine,
                            ins=[], outs=[],
                            sync_info=mybir.SyncInfo(on_wait=[w],
                                                     on_update=[]),
                        )
                        nc.register_instruction(carrier)
                        out.append(carrier)
                    inst.sync_info = mybir.SyncInfo(
                        on_wait=[waits[-1]],
                        on_update=list(si.on_update) if si.on_update else [])
                    changed = True
                out.append(inst)
            if changed:
                blk.instructions = out


_MODULES = {}


def _get_module(npair: int):
    if npair not in _MODULES:
        nc = _build_module(npair)
        _legalize_multiwait(nc)
        _MODULES[npair] = nc
    return _MODULES[npair]


def run(board_free, filters, areas, trace=False, **spmd_kwargs):
    from concourse.bass_utils import run_bass_kernel_spmd

    c0, c1, c1v = _plan_cols(filters)
    npair = len(c0)
    boardt = _build_boardt(board_free)
    mmat = _build_m(filters, c0, c1, c1v)

    in_maps = [
        {"boardt": boardt[c], "mmat": mmat}
        for c in range(N_CORES)
    ]
    nc = _get_module(npair)
    res = run_bass_kernel_spmd(nc, in_maps, core_ids=list(range(N_CORES)),
                               trace=trace, **spmd_kwargs)
    # device layout [128, (block, col)] -> [core*block*board, col]
    A = np.concatenate(
        [np.asarray(r["out"]).reshape(128, NKB, npair).transpose(1, 0, 2)
         for r in res.results],
        axis=0).reshape(BATCH, npair).astype(np.int32)  # exact ints <= 825

    ar = np.asarray(areas, dtype=np.int32).reshape(NF)
    a0 = ar[c0 // NPOS]
    corr0 = A & (PACK - 1)
    corr1 = A >> 5
    out = np.zeros((BATCH, NCOL), dtype=np.float32)
    out[:, c0] = (corr0 == a0[None, :]).astype(np.float32)
    a1 = ar[c1[c1v] // NPOS]
    out[:, c1[c1v]] = (corr1[:, c1v] == a1[None, :]).astype(np.float32)
    return out.reshape(BATCH, NF, 9, 9), res


def kernel(board_free, filters, areas):
    out, _ = run(board_free, filters, areas)
    return out
